# revision 20
# baseline (speedup 1.0000x reference)
"""Trainium2 Bass kernel for nn_EnergyFunction (8-core SPMD).

Reference computation (per batch b):
    Q = features @ Wq;  K = features @ Wk                     # [S, 64]
    scores = (Q @ K.T) / 8 * locality_scale / max(|i-j|, 1)   # [S, S]
    charge = sigmoid(features @ w_charge + b_charge)          # [S]
    energy = -scores * charge_i * charge_j

Sharding: core = (b, i-half). Each of the 8 cores handles one batch b
(= core // 2) and one half of the query rows (i0 = (core % 2) * 2048),
producing a [2048, 4096] block of the [4, 4096, 4096] output.

Device-side plan (per core):
  - Inputs in fp16 (features pre-transposed to [512, S] feature-major on
    the host; projection weights [Wk|w_charge] / [Wq*(-loc/8)|w_charge]).
  - Prelim per 512-col seg: 4 accumulating fp16 matmuls -> psum [65,512]
    (rows 0:64 = X^T, row 64 = charge logits); ACT sigmoid -> charge row;
    gpsimd partition_broadcast replicates the charge row to SBUF; one DVE
    multiply folds it straight out of PSUM: K'^T = K^T * c_j (fp16),
    Q'^T = Q^T * c_i (fp16).
  - Main loop (16 i-tiles x 4 j-blocks, j-outer): 2x PE matmul fp16
    [64c,128m,512n] into a 2-bank psum [128, 1024]; the Toeplitz-mask
    multiply + fp16 downcast (the PSUM drain) is split across three
    engines by a static per-tile pattern: V = DVE direct, A = ACT copy
    to fp16 SBUF + DVE fp16 multiply, P = gpsimd direct. 512 KB -> 256 KB
    fp16 DMA out per tile; host upcasts to fp32.
    K-side prelim groups are prefetched one j-block ahead.
    Mask band: vb2d[p, u] = 1/max(|i_base + 1920 + p - u|, 1)
    (host input [128, 6016]; tile (t, j) uses u0 = 1024 j - 128 t + 1920).
"""

import numpy as np

import concourse.bacc as bacc
import concourse.mybir as mybir
from concourse import tile
from concourse import bass_utils

# Problem shape (hardcoded per harness contract)
B = 4
S = 4096
F = 512
D = 64

P = 128            # partition tile (i)
SEG = 512          # j segment width (one PSUM bank of fp32)
WOUT = 1024        # epilogue / output tile width (2 PSUM banks)
IHALF = S // 2     # 2048 query rows per core
NIT = IHALF // P   # 16 i-tiles
NSEG = S // SEG    # 8 j segments
NJP = S // WOUT    # 4 j output tiles per i-tile
NQSEG = IHALF // SEG  # 4 q segments
NCH = F // P       # 4 feature chunks
C0 = IHALF - P     # 1920 mask-band column offset
MBW = (S - SEG) + C0 + SEG  # 6016 mask band width

F32 = mybir.dt.float32
F16 = mybir.dt.float16
SIG = mybir.ActivationFunctionType.Sigmoid
COPY = mybir.ActivationFunctionType.Copy

# Drain-path pattern over the 64 output tiles (index = j * NIT + t):
# 'V' = DVE direct from PSUM (1.43 ns/col measured);
# 'P' = ACT copy to SBUF + gpsimd fp16 mul (gpsimd cannot read PSUM;
#       ACT 1.30 + Pool 2.48 ns/col measured, on different engines).
# 43 V / 21 P balances DVE(+folds) against Pool(+broadcasts) at ~70us.
PATTERN = ['P' if i % 3 == 1 else 'V' for i in range(64)]

_PROGRAM = None


def _build_program():
    nc = bacc.Bacc("TRN2", target_bir_lowering=False, debug=False, num_devices=8)

    fK = nc.dram_tensor("fK", [F, S], F16, kind="ExternalInput").ap()
    fQ = nc.dram_tensor("fQ", [F, IHALF], F16, kind="ExternalInput").ap()
    # [Wk | w_charge] and [Wq * (-loc/8) | w_charge], both [F, 65]
    wk65 = nc.dram_tensor("wk65", [F, D + 1], F16, kind="ExternalInput").ap()
    wq65 = nc.dram_tensor("wq65", [F, D + 1], F16, kind="ExternalInput").ap()
    bvec = nc.dram_tensor("bvec", [P, 1], F32, kind="ExternalInput").ap()
    vb2d = nc.dram_tensor("vb2d", [P, MBW], F16, kind="ExternalInput").ap()
    energy = nc.dram_tensor("energy", [IHALF, S], F16, kind="ExternalOutput").ap()

    W65 = D + 1
    NSH = WOUT // SEG      # matmul halves per output tile
    VBC = 4                # mask band load chunks
    VBW = MBW // VBC       # 1504

    with tile.TileContext(nc) as tc:
        with (
            tc.tile_pool(name="const", bufs=1) as const,
            tc.tile_pool(name="stage", bufs=1) as stage,
        ):
            bvec_sb = const.tile([P, 1], F32, tag="bvec")
            nc.sync.dma_start(out=bvec_sb[:], in_=bvec)
            wk_sb = const.tile([P, NCH * W65], F16, tag="wk")
            wq_sb = const.tile([P, NCH * W65], F16, tag="wq")
            for c in range(NCH):
                nc.sync.dma_start(
                    out=wk_sb[:, c * W65:(c + 1) * W65],
                    in_=wk65[c * P:(c + 1) * P, :],
                )
                nc.sync.dma_start(
                    out=wq_sb[:, c * W65:(c + 1) * W65],
                    in_=wq65[c * P:(c + 1) * P, :],
                )

            # Persistent prelim outputs (fp16 so the main matmuls run at
            # 1 cycle/col and weight loads move half the bytes)
            QT = stage.tile([D, IHALF], F16, tag="qt")     # Q^T * c_i
            KpT = stage.tile([D, S], F16, tag="kpt")       # K^T * c_j
            crow = stage.tile([1, S], F16, tag="crow")     # K-side charge row
            qrow = stage.tile([1, IHALF], F16, tag="qrow")  # Q-side charge row
            vb_sb = stage.tile([P, MBW], F16, tag="vb")

            with (
                tc.tile_pool(name="feat", bufs=1) as fpool,
                tc.tile_pool(name="pp", space="PSUM", bufs=2) as ps_p,
            ):
                fk = [fpool.tile([P, S], F16, tag=f"fk{c}", name=f"fkt{c}")
                      for c in range(NCH)]
                fq = [fpool.tile([P, IHALF], F16, tag=f"fq{c}", name=f"fqt{c}")
                      for c in range(NCH)]

                def _load_fq_half(half):
                    lo, hi = half * (IHALF // 2), (half + 1) * (IHALF // 2)
                    for c in range(NCH):
                        nc.sync.dma_start(
                            out=fq[c][:, lo:hi], in_=fQ[c * P:(c + 1) * P, lo:hi]
                        )

                def _load_fk_block(b):
                    lo, hi = b * 1024, (b + 1) * 1024
                    for c in range(NCH):
                        nc.sync.dma_start(
                            out=fk[c][:, lo:hi], in_=fK[c * P:(c + 1) * P, lo:hi]
                        )

                def _load_vb(v):
                    nc.sync.dma_start(
                        out=vb_sb[:, v * VBW:(v + 1) * VBW],
                        in_=vb2d[:, v * VBW:(v + 1) * VBW],
                    )

                # Early inputs (feeding prelim + first drains) go on the
                # Sync DMA queue ahead of the output stream; late inputs
                # are dispatched from the Scalar engine's separate DMA
                # queue (emitted between prelim groups below) so they
                # never sit ahead of output tiles in the Sync FIFO.
                _load_fk_block(0)
                _load_fq_half(0)
                _load_fq_half(1)
                _load_vb(1)
                _load_fk_block(1)

                def _load_late(name):
                    if name[0] == 'v':
                        v = int(name[1])
                        nc.scalar.dma_start(
                            out=vb_sb[:, v * VBW:(v + 1) * VBW],
                            in_=vb2d[:, v * VBW:(v + 1) * VBW],
                        )
                    else:
                        b = int(name[1])
                        lo, hi = b * 1024, (b + 1) * 1024
                        for c in range(NCH):
                            nc.scalar.dma_start(
                                out=fk[c][:, lo:hi],
                                in_=fK[c * P:(c + 1) * P, lo:hi],
                            )

                # Per-seg projection chain: 4 accumulating matmuls ->
                # ACT sigmoid (charge row) -> gpsimd broadcast to SBUF ->
                # one DVE multiply folds the charge straight out of PSUM
                # into fp16 Q'/K'.
                def _emit_group(side, s):
                    w_sb = wk_sb if side == "k" else wq_sb
                    f_t = fk if side == "k" else fq
                    row = crow if side == "k" else qrow
                    dst = KpT if side == "k" else QT
                    pX = ps_p.tile([W65, SEG], F32, tag="pp")
                    for c in range(NCH):
                        nc.tensor.matmul(
                            pX[:],
                            w_sb[:, c * W65:(c + 1) * W65],
                            f_t[c][:, s * SEG:(s + 1) * SEG],
                            start=(c == 0),
                            stop=(c == NCH - 1),
                        )
                    nc.scalar.activation(
                        row[0:1, s * SEG:(s + 1) * SEG], pX[D:D + 1, :],
                        SIG, bias=bvec_sb[0:1, :], scale=1.0,
                    )
                    # Stage X^T out of PSUM on the scalar engine so the pX
                    # slot recycles at ACT pace; the fold then runs all-SBUF
                    # and can lag freely on the busy DVE queue without
                    # stalling the PE prelim matmuls.
                    xs = stage.tile([D, SEG], F16, tag="xs", bufs=3)
                    nc.scalar.activation(xs[:], pX[0:D, :], COPY)
                    Cb = stage.tile([D, SEG], F16, tag="cb", bufs=2)
                    nc.gpsimd.partition_broadcast(
                        Cb[:], row[0:1, s * SEG:(s + 1) * SEG]
                    )
                    nc.vector.tensor_mul(
                        out=dst[:, s * SEG:(s + 1) * SEG],
                        in0=xs[:],
                        in1=Cb[:],
                    )

                # Upfront: k-groups for the first TWO j-blocks plus the
                # whole q side; later k-groups are prefetched one block
                # ahead inside the main loop. k2/k3 are interleaved early
                # so their ACT/Pool/DVE chains finish before the main
                # loop's drain work floods those queues.
                _emit_group("k", 0)
                _emit_group("k", 1)
                _emit_group("q", 0)
                _load_late("v0")
                _emit_group("q", 1)
                _load_late("v2")
                _emit_group("k", 2)
                _load_late("f2")
                _emit_group("q", 2)
                _load_late("f3")
                _emit_group("q", 3)
                _load_late("v3")
                _emit_group("k", 3)

                with (
                    tc.tile_pool(name="pse", space="PSUM", bufs=3) as ps_e,
                    tc.tile_pool(name="osb", bufs=14) as opool,
                    tc.tile_pool(name="atmp", bufs=6) as apool,
                ):
                    for j in range(NJP):
                        for t in range(NIT):
                            # Prefetch next block's K prelim groups mid-block
                            # so the PE/ACT/Pool/DVE work they need is spread
                            # between tile chains instead of bursting at the
                            # block boundary (which starves the drain
                            # engines and drops the PE out of max clock).
                            if 1 <= j < NJP - 1:
                                if t == 4:
                                    _emit_group("k", 2 * (j + 1))
                                elif t == 10:
                                    _emit_group("k", 2 * (j + 1) + 1)
                            pe_ = ps_e.tile([P, WOUT], F32)
                            for h in range(NSH):
                                nc.tensor.matmul(
                                    pe_[:, h * SEG:(h + 1) * SEG],
                                    QT[:, t * P:(t + 1) * P],
                                    KpT[:, (NSH * j + h) * SEG:
                                        (NSH * j + h + 1) * SEG],
                                    start=True,
                                    stop=True,
                                )
                            osb = opool.tile([P, WOUT], F16)
                            u0 = j * WOUT - t * P + C0
                            path = PATTERN[(j * NIT + t) % len(PATTERN)]
                            if path == 'V':
                                nc.vector.tensor_mul(
                                    out=osb[:], in0=pe_[:],
                                    in1=vb_sb[:, u0:u0 + WOUT],
                                )
                            else:
                                atmp = apool.tile([P, WOUT], F16)
                                nc.scalar.activation(atmp[:], pe_[:], COPY)
                                eng = nc.vector if path == 'A' else nc.gpsimd
                                eng.tensor_mul(
                                    out=osb[:], in0=atmp[:],
                                    in1=vb_sb[:, u0:u0 + WOUT],
                                )
                            nc.sync.dma_start(
                                out=energy[t * P:(t + 1) * P,
                                           j * WOUT:(j + 1) * WOUT],
                                in_=osb[:],
                            )

    nc.compile()
    return nc


def _get_program():
    global _PROGRAM
    if _PROGRAM is None:
        _PROGRAM = _build_program()
    return _PROGRAM


def _make_in_maps(features, Wq, Wk, w_charge, b_charge, loc):
    wq_s = Wq * np.float32(-loc / 8.0)
    wq65 = np.ascontiguousarray(
        np.concatenate([wq_s, w_charge[:, None]], axis=1).astype(np.float16)
    )
    wk65 = np.ascontiguousarray(
        np.concatenate([Wk, w_charge[:, None]], axis=1).astype(np.float16)
    )
    bvec = np.full((P, 1), b_charge, dtype=np.float32)

    u = np.arange(MBW, dtype=np.float32)[None, :]
    vb_half = []
    for h in range(2):
        ib = (h * IHALF + C0 + np.arange(P, dtype=np.float32))[:, None]
        vb_half.append(np.ascontiguousarray(
            (1.0 / np.maximum(np.abs(ib - u), 1.0)).astype(np.float16)
        ))

    fT = [np.ascontiguousarray(features[b].T.astype(np.float16)) for b in range(B)]

    in_maps = []
    for core in range(2 * B):
        b, h = divmod(core, 2)
        i0 = h * IHALF
        in_maps.append({
            "fK": fT[b],
            "fQ": np.ascontiguousarray(fT[b][:, i0:i0 + IHALF]),
            "wk65": wk65,
            "wq65": wq65,
            "bvec": bvec,
            "vb2d": vb_half[h],
        })
    return in_maps


def kernel(features, Wq, Wk, w_charge, b_charge, locality_scale):
    features = np.asarray(features, dtype=np.float32)
    Wq = np.asarray(Wq, dtype=np.float32)
    Wk = np.asarray(Wk, dtype=np.float32)
    w_charge = np.asarray(w_charge, dtype=np.float32)
    b_charge = float(np.asarray(b_charge))
    loc = float(np.asarray(locality_scale))

    nc = _get_program()
    in_maps = _make_in_maps(features, Wq, Wk, w_charge, b_charge, loc)
    res = bass_utils.run_bass_kernel_spmd(nc, in_maps, core_ids=list(range(2 * B)))

    out = np.empty((B, S, S), dtype=np.float32)
    for core in range(2 * B):
        b, h = divmod(core, 2)
        out[b, h * IHALF:(h + 1) * IHALF, :] = res.results[core]["energy"]
    return out


# revision 24
# speedup vs baseline: 1.6715x; 1.6715x over previous
"""Trainium2 Bass kernel for nn_EnergyFunction (8-core SPMD).

Reference computation (per batch b):
    Q = features @ Wq;  K = features @ Wk                     # [S, 64]
    scores = (Q @ K.T) / 8 * locality_scale / max(|i-j|, 1)   # [S, S]
    charge = sigmoid(features @ w_charge + b_charge)          # [S]
    energy = -scores * charge_i * charge_j

Sharding: core = (b, i-half). Each of the 8 cores handles one batch b
(= core // 2) and one half of the query rows (i0 = (core % 2) * 2048),
producing a [2048, 4096] block of the [4, 4096, 4096] output.

Device-side plan (per core):
  - Inputs in fp16 (features pre-transposed to [512, S] feature-major on
    the host; projection weights [Wk|w_charge] / [Wq*(-loc/8)|w_charge]).
  - Prelim per 512-col seg: 4 accumulating fp16 matmuls -> psum [65,512]
    (rows 0:64 = X^T, row 64 = charge logits); ACT sigmoid -> charge row;
    gpsimd partition_broadcast replicates the charge row to SBUF; one DVE
    multiply folds it straight out of PSUM: K'^T = K^T * c_j (fp16),
    Q'^T = Q^T * c_i (fp16).
  - Main loop (16 i-tiles x 4 j-blocks, j-outer): 2x PE matmul fp16
    [64c,128m,512n] into a 2-bank psum [128, 1024]; the Toeplitz-mask
    multiply + fp16 downcast (the PSUM drain) is split across three
    engines by a static per-tile pattern: V = DVE direct, A = ACT copy
    to fp16 SBUF + DVE fp16 multiply, P = gpsimd direct. 512 KB -> 256 KB
    fp16 DMA out per tile; host upcasts to fp32.
    K-side prelim groups are prefetched one j-block ahead.
    Mask band: vb2d[p, u] = 1/max(|i_base + 1920 + p - u|, 1)
    (host input [128, 6016]; tile (t, j) uses u0 = 1024 j - 128 t + 1920).
"""

import numpy as np

import concourse.bacc as bacc
import concourse.mybir as mybir
from concourse import tile
from concourse import bass_utils

# Problem shape (hardcoded per harness contract)
B = 4
S = 4096
F = 512
D = 64

P = 128            # partition tile (i)
SEG = 512          # j segment width (one PSUM bank of fp32)
WOUT = 1024        # epilogue / output tile width (2 PSUM banks)
IHALF = S // 2     # 2048 query rows per core
NIT = IHALF // P   # 16 i-tiles
NSEG = S // SEG    # 8 j segments
NJP = S // WOUT    # 4 j output tiles per i-tile
NQSEG = IHALF // SEG  # 4 q segments
NCH = F // P       # 4 feature chunks
C0 = IHALF - P     # 1920 mask-band column offset
MBW = (S - SEG) + C0 + SEG  # 6016 mask band width

F32 = mybir.dt.float32
F16 = mybir.dt.float16
SIG = mybir.ActivationFunctionType.Sigmoid
COPY = mybir.ActivationFunctionType.Copy

# Drain-path pattern over the 64 output tiles (index = j * NIT + t):
# 'V' = DVE direct from PSUM (1.43 ns/col measured);
# 'P' = ACT copy to SBUF + gpsimd fp16 mul (gpsimd cannot read PSUM;
#       ACT 1.30 + Pool 2.48 ns/col measured, on different engines).
# 43 V / 21 P balances DVE(+folds) against Pool(+broadcasts) at ~70us.
PATTERN = ['V' for i in range(64)]

_PROGRAM = None


def _build_program():
    nc = bacc.Bacc("TRN2", target_bir_lowering=False, debug=False, num_devices=8)

    fK = nc.dram_tensor("fK", [F, S], F16, kind="ExternalInput").ap()
    fQ = nc.dram_tensor("fQ", [F, IHALF], F16, kind="ExternalInput").ap()
    # [Wk | w_charge] and [Wq * (-loc/8) | w_charge], both [F, 65]
    wk65 = nc.dram_tensor("wk65", [F, D + 1], F16, kind="ExternalInput").ap()
    wq65 = nc.dram_tensor("wq65", [F, D + 1], F16, kind="ExternalInput").ap()
    bvec = nc.dram_tensor("bvec", [P, 1], F32, kind="ExternalInput").ap()
    vb2d = nc.dram_tensor("vb2d", [P, MBW], F16, kind="ExternalInput").ap()
    energy = nc.dram_tensor("energy", [IHALF, S], F16, kind="ExternalOutput").ap()

    W65 = D + 1
    NSH = WOUT // SEG      # matmul halves per output tile
    VBC = 4                # mask band load chunks
    VBW = MBW // VBC       # 1504

    with tile.TileContext(nc) as tc:
        with (
            tc.tile_pool(name="const", bufs=1) as const,
            tc.tile_pool(name="stage", bufs=1) as stage,
        ):
            bvec_sb = const.tile([P, 1], F32, tag="bvec")
            nc.sync.dma_start(out=bvec_sb[:], in_=bvec)
            wk_sb = const.tile([P, NCH * W65], F16, tag="wk")
            wq_sb = const.tile([P, NCH * W65], F16, tag="wq")
            for c in range(NCH):
                nc.sync.dma_start(
                    out=wk_sb[:, c * W65:(c + 1) * W65],
                    in_=wk65[c * P:(c + 1) * P, :],
                )
                nc.sync.dma_start(
                    out=wq_sb[:, c * W65:(c + 1) * W65],
                    in_=wq65[c * P:(c + 1) * P, :],
                )

            # Persistent prelim outputs (fp16 so the main matmuls run at
            # 1 cycle/col and weight loads move half the bytes)
            QT = stage.tile([D, IHALF], F16, tag="qt")     # Q^T * c_i
            KpT = stage.tile([D, S], F16, tag="kpt")       # K^T * c_j
            crow = stage.tile([1, S], F16, tag="crow")     # K-side charge row
            qrow = stage.tile([1, IHALF], F16, tag="qrow")  # Q-side charge row
            vb_sb = stage.tile([P, MBW], F16, tag="vb")

            with (
                tc.tile_pool(name="feat", bufs=1) as fpool,
                tc.tile_pool(name="pp", space="PSUM", bufs=2) as ps_p,
            ):
                fk = [fpool.tile([P, S], F16, tag=f"fk{c}", name=f"fkt{c}")
                      for c in range(NCH)]
                fq = [fpool.tile([P, IHALF], F16, tag=f"fq{c}", name=f"fqt{c}")
                      for c in range(NCH)]

                def _load_fq_half(half):
                    lo, hi = half * (IHALF // 2), (half + 1) * (IHALF // 2)
                    for c in range(NCH):
                        nc.sync.dma_start(
                            out=fq[c][:, lo:hi], in_=fQ[c * P:(c + 1) * P, lo:hi]
                        )

                def _load_fk_block(b):
                    lo, hi = b * 1024, (b + 1) * 1024
                    for c in range(NCH):
                        nc.sync.dma_start(
                            out=fk[c][:, lo:hi], in_=fK[c * P:(c + 1) * P, lo:hi]
                        )

                def _load_vb(v):
                    nc.sync.dma_start(
                        out=vb_sb[:, v * VBW:(v + 1) * VBW],
                        in_=vb2d[:, v * VBW:(v + 1) * VBW],
                    )

                # Early inputs (feeding prelim + first drains) go on the
                # Sync DMA queue ahead of the output stream; late inputs
                # are dispatched from the Scalar engine's separate DMA
                # queue (emitted between prelim groups below) so they
                # never sit ahead of output tiles in the Sync FIFO.
                _load_fk_block(0)
                _load_fq_half(0)
                _load_fq_half(1)
                _load_vb(1)
                _load_fk_block(1)
                _load_vb(0)
                _load_vb(2)
                _load_fk_block(2)
                _load_fk_block(3)
                _load_vb(3)

                # Per-seg projection chain: 4 accumulating matmuls ->
                # ACT sigmoid (charge row) -> gpsimd broadcast to SBUF ->
                # one DVE multiply folds the charge straight out of PSUM
                # into fp16 Q'/K'.
                def _emit_group(side, s):
                    w_sb = wk_sb if side == "k" else wq_sb
                    f_t = fk if side == "k" else fq
                    row = crow if side == "k" else qrow
                    dst = KpT if side == "k" else QT
                    pX = ps_p.tile([W65, SEG], F32, tag="pp")
                    for c in range(NCH):
                        nc.tensor.matmul(
                            pX[:],
                            w_sb[:, c * W65:(c + 1) * W65],
                            f_t[c][:, s * SEG:(s + 1) * SEG],
                            start=(c == 0),
                            stop=(c == NCH - 1),
                        )
                    nc.scalar.activation(
                        row[0:1, s * SEG:(s + 1) * SEG], pX[D:D + 1, :],
                        SIG, bias=bvec_sb[0:1, :], scale=1.0,
                    )
                    # Stage X^T out of PSUM on the scalar engine so the pX
                    # slot recycles at ACT pace; the fold then runs all-SBUF
                    # and can lag freely on the busy DVE queue without
                    # stalling the PE prelim matmuls.
                    xs = stage.tile([D, SEG], F16, tag="xs", bufs=3)
                    nc.scalar.activation(xs[:], pX[0:D, :], COPY)
                    Cb = stage.tile([D, SEG], F16, tag="cb", bufs=2)
                    nc.gpsimd.partition_broadcast(
                        Cb[:], row[0:1, s * SEG:(s + 1) * SEG]
                    )
                    nc.vector.tensor_mul(
                        out=dst[:, s * SEG:(s + 1) * SEG],
                        in0=xs[:],
                        in1=Cb[:],
                    )

                # Upfront: k-groups for the first TWO j-blocks plus the
                # whole q side; later k-groups are prefetched one block
                # ahead inside the main loop. k2/k3 are interleaved early
                # so their ACT/Pool/DVE chains finish before the main
                # loop's drain work floods those queues.
                _emit_group("k", 0)
                _emit_group("k", 1)
                _emit_group("q", 0)
                _emit_group("q", 1)
                _emit_group("k", 2)
                _emit_group("q", 2)
                _emit_group("q", 3)
                _emit_group("k", 3)

                with (
                    tc.tile_pool(name="pse", space="PSUM", bufs=3) as ps_e,
                    tc.tile_pool(name="osb", bufs=8) as opool,
                    tc.tile_pool(name="atmp", bufs=4) as apool,
                ):
                    for j in range(NJP):
                        for t in range(NIT):
                            # Prefetch next block's K prelim groups mid-block
                            # so the PE/ACT/Pool/DVE work they need is spread
                            # between tile chains instead of bursting at the
                            # block boundary (which starves the drain
                            # engines and drops the PE out of max clock).
                            if 1 <= j < NJP - 1:
                                if t == 4:
                                    _emit_group("k", 2 * (j + 1))
                                elif t == 10:
                                    _emit_group("k", 2 * (j + 1) + 1)
                            pe_ = ps_e.tile([P, WOUT], F32)
                            for h in range(NSH):
                                nc.tensor.matmul(
                                    pe_[:, h * SEG:(h + 1) * SEG],
                                    QT[:, t * P:(t + 1) * P],
                                    KpT[:, (NSH * j + h) * SEG:
                                        (NSH * j + h + 1) * SEG],
                                    start=True,
                                    stop=True,
                                )
                            osb = opool.tile([P, WOUT], F16)
                            u0 = j * WOUT - t * P + C0
                            path = PATTERN[(j * NIT + t) % len(PATTERN)]
                            if path == 'V':
                                nc.vector.tensor_mul(
                                    out=osb[:], in0=pe_[:],
                                    in1=vb_sb[:, u0:u0 + WOUT],
                                )
                            else:
                                atmp = apool.tile([P, WOUT], F16)
                                nc.scalar.activation(atmp[:], pe_[:], COPY)
                                eng = nc.vector if path == 'A' else nc.gpsimd
                                eng.tensor_mul(
                                    out=osb[:], in0=atmp[:],
                                    in1=vb_sb[:, u0:u0 + WOUT],
                                )
                            nc.sync.dma_start(
                                out=energy[t * P:(t + 1) * P,
                                           j * WOUT:(j + 1) * WOUT],
                                in_=osb[:],
                            )

    nc.compile()
    return nc


def _get_program():
    global _PROGRAM
    if _PROGRAM is None:
        _PROGRAM = _build_program()
    return _PROGRAM


def _make_in_maps(features, Wq, Wk, w_charge, b_charge, loc):
    wq_s = Wq * np.float32(-loc / 8.0)
    wq65 = np.ascontiguousarray(
        np.concatenate([wq_s, w_charge[:, None]], axis=1).astype(np.float16)
    )
    wk65 = np.ascontiguousarray(
        np.concatenate([Wk, w_charge[:, None]], axis=1).astype(np.float16)
    )
    bvec = np.full((P, 1), b_charge, dtype=np.float32)

    u = np.arange(MBW, dtype=np.float32)[None, :]
    vb_half = []
    for h in range(2):
        ib = (h * IHALF + C0 + np.arange(P, dtype=np.float32))[:, None]
        vb_half.append(np.ascontiguousarray(
            (1.0 / np.maximum(np.abs(ib - u), 1.0)).astype(np.float16)
        ))

    fT = [np.ascontiguousarray(features[b].T.astype(np.float16)) for b in range(B)]

    in_maps = []
    for core in range(2 * B):
        b, h = divmod(core, 2)
        i0 = h * IHALF
        in_maps.append({
            "fK": fT[b],
            "fQ": np.ascontiguousarray(fT[b][:, i0:i0 + IHALF]),
            "wk65": wk65,
            "wq65": wq65,
            "bvec": bvec,
            "vb2d": vb_half[h],
        })
    return in_maps


def kernel(features, Wq, Wk, w_charge, b_charge, locality_scale):
    features = np.asarray(features, dtype=np.float32)
    Wq = np.asarray(Wq, dtype=np.float32)
    Wk = np.asarray(Wk, dtype=np.float32)
    w_charge = np.asarray(w_charge, dtype=np.float32)
    b_charge = float(np.asarray(b_charge))
    loc = float(np.asarray(locality_scale))

    nc = _get_program()
    in_maps = _make_in_maps(features, Wq, Wk, w_charge, b_charge, loc)
    res = bass_utils.run_bass_kernel_spmd(nc, in_maps, core_ids=list(range(2 * B)))

    out = np.empty((B, S, S), dtype=np.float32)
    for core in range(2 * B):
        b, h = divmod(core, 2)
        out[b, h * IHALF:(h + 1) * IHALF, :] = res.results[core]["energy"]
    return out
